# revision 19
# baseline (speedup 1.0000x reference)
"""Tucker-style 3-mode contraction kernel for Trainium2 (8 NeuronCores).

Problem: x [1024*32*32*32] fp32, w0/w1/w2 [32,32] fp32.
  out[B,A,Bb,C] = sum_{a,b,c} x[B,a,b,c] w0[a,A] w1[b,Bb] w2[c,C]

v9: bf16 I/O (host casts), contract order a -> c -> b, 2-sub-tile macros.
Per core: 128 batch elems as 32 sub-tiles of [128 p = (g4, mode32), 1024 f],
processed as 16 macros of 2 sub-tiles. Stationary weights kron(I4, w)
[128,128] bf16; stage-major MM bursts (4 same-weight MMs) + walrus
ldw-opt so LDWEIGHTS dedups and MMs pipeline back-to-back.

Per macro (2 sub-tiles, free slabs t=0,1):
  MM1 wk0 x4 -> z1 [(g,A),(t,b,c)] f32   [128,2048] psum (4 banks)
  T1  DVE f32 stream-transpose PSUM->SBUF -> t1 [(g,c),(t,b,A)] f32
  MM2 wk2 x4, rhs = high bf16 halves of t1 (stride-2 AP, truncation cast)
      -> z2 [(g,C),(t,b,A)] f32
  E2  ACT reorder+cast -> t2 [(g,C),(t,Ah,b,Ap)] bf16   (A = 2*Ah+Ap)
  T2  DVE u32-pair transpose -> t2t [(g,b),(t,Ah,C,Ap)] bf16
  MM3 wk1 x4 -> z3 [(g,B),(t,Ah,C,Ap)] f32
  E3  cast evac -> Y bf16 (ACT cols 0:E3A, DVE rest)
  PSUM: one pool, 2 bufs x [128,2048]; z1/z2/z3 cycle through it.
"""

import os

import numpy as np

N_CORES = 8
BATCH = 1024
F = 32
FF = F * F  # 1024
ELEM = F * FF  # 32768
B_PER_CORE = BATCH // N_CORES  # 128
G = 4  # batch groups per sub-tile
NT = B_PER_CORE // G  # 32 sub-tiles per core
SS = 4  # sub-tiles per super-tile (DMA batch)
NST = NT // SS  # 8 super-tiles per core
M = 2  # sub-tiles per macro
FM = M * FF  # 2048

LDW_OPT = os.environ.get("KERNEL_LDW_OPT", "0") == "1"
E3_DVE = int(os.environ.get("KERNEL_E3_DVE", "512"))  # cols on DVE (of FM)

X_DTYPE = Z_DTYPE = "bfloat16"  # for test.py printout compat

_CACHE = {}
_PATCHED = False


def _patch_ldw_opt():
    """walrus --enable-ldw-opt=false is hardcoded in bass_utils; flip it so
    consecutive same-weight matmuls dedup their LDWEIGHTS."""
    global _PATCHED
    if _PATCHED or not LDW_OPT:
        return
    import concourse.bass_utils as bu
    import concourse.bacc as bacc
    orig = bu.run_command

    def patched(cmd, *a, **kw):
        if isinstance(cmd, list):
            cmd = ["--enable-ldw-opt=true" if c == "--enable-ldw-opt=false"
                   else c for c in cmd]
        return orig(cmd, *a, **kw)

    bu.run_command = patched
    # waits attached to Ldweights make walrus's LDW dedup refuse; keep the
    # waits on the matmuls instead (generate_event_semaphores splits >1).
    bacc.Bacc.move_matmul_waits_to_ldweights = lambda self: None
    _PATCHED = True


def build_program():
    key = (LDW_OPT, E3_DVE)
    if key in _CACHE:
        return _CACHE[key]
    _patch_ldw_opt()

    import concourse.bacc as bacc
    import concourse.mybir as mybir
    import concourse.tile as tile

    f32 = mybir.dt.float32
    u32 = mybir.dt.uint32
    bf16 = mybir.dt.bfloat16

    nc = bacc.Bacc("TRN2", target_bir_lowering=False, debug=False,
                   num_devices=N_CORES)

    xs = nc.dram_tensor("xs", [NT, 128, FF], bf16, kind="ExternalInput")
    wk0 = nc.dram_tensor("wk0", [128, 128], bf16, kind="ExternalInput")
    wk1 = nc.dram_tensor("wk1", [128, 128], bf16, kind="ExternalInput")
    wk2 = nc.dram_tensor("wk2", [128, 128], bf16, kind="ExternalInput")
    ys = nc.dram_tensor("ys", [NT, 128, FF], bf16, kind="ExternalOutput")

    def mm(out_ap, lhsT_ap, rhs_ap):
        nc.tensor.matmul(out_ap, lhsT_ap, rhs_ap, start=True, stop=True)

    with tile.TileContext(nc) as tc:
        with (
            tc.tile_pool(name="consts", bufs=1) as cpool,
            tc.tile_pool(name="xp", bufs=3) as xp,
            tc.tile_pool(name="t1p", bufs=2) as t1p,
            tc.tile_pool(name="t2p", bufs=2) as t2p,
            tc.tile_pool(name="t2tp", bufs=2) as t2tp,
            tc.tile_pool(name="yp", bufs=2) as yp,
            tc.tile_pool(name="psA", bufs=2, space="PSUM") as psA,
        ):
            wk0t = cpool.tile([128, 128], bf16)
            wk1t = cpool.tile([128, 128], bf16)
            wk2t = cpool.tile([128, 128], bf16)
            nc.sync.dma_start(out=wk0t[:], in_=wk0[:])
            nc.sync.dma_start(out=wk1t[:], in_=wk1[:])
            nc.sync.dma_start(out=wk2t[:], in_=wk2[:])

            for st in range(NST):
                X = xp.tile([128, SS, FF], bf16, tag="X")
                nc.sync.dma_start(
                    out=X[:],
                    in_=xs[st * SS:(st + 1) * SS].rearrange("t p f -> p t f"))
                Y = yp.tile([128, SS, FF], bf16, tag="Y")
                for m in range(SS // M):
                    s0 = m * M  # first sub-tile of macro within super-tile
                    # MM1 burst: contract a -> z1 [(g,A),(t,b,c)]
                    z1 = psA.tile([128, FM], f32, tag="z")
                    for t in range(M):
                        o = t * FF
                        mm(z1[:, o:o + 512], wk0t[:], X[:, s0 + t, 0:512])
                        mm(z1[:, o + 512:o + 1024], wk0t[:],
                           X[:, s0 + t, 512:1024])
                    # T1: f32 psum->sbuf transpose -> [(g,c),(t,b,A)] f32
                    t1 = t1p.tile([128, FM], f32, tag="t1")
                    nc.vector.transpose(out=t1[:], in_=z1[:])
                    # MM2 burst: rhs = high bf16 halves (truncation cast)
                    t1v = t1[:].bitcast(bf16).rearrange(
                        "p (t b a two) -> p t b a two",
                        t=M, b=F, a=F, two=2)
                    z2 = psA.tile([128, FM], f32, tag="z")
                    for t in range(M):
                        o = t * FF
                        mm(z2[:, o:o + 512], wk2t[:], t1v[:, t, 0:16, :, 1])
                        mm(z2[:, o + 512:o + 1024], wk2t[:],
                           t1v[:, t, 16:32, :, 1])
                    # E2: reorder+cast -> t2 [(g,C),(t,Ah,b,Ap)]
                    # (per-tile ops: ISA mem patterns cap at 3 free dims)
                    t2 = t2p.tile([128, M, 16, F, 2], bf16, tag="t2")
                    for t in range(M):
                        nc.scalar.copy(
                            out=t2[:, t],
                            in_=z2[:, t * FF:(t + 1) * FF].rearrange(
                                "p (b ah ap) -> p ah b ap",
                                b=F, ah=16, ap=2))
                    # T2: u32 pair transpose -> [(g,b),(t,Ah,C,Ap)]
                    t2t = t2tp.tile([128, M * 512], u32, tag="t2t")
                    nc.vector.transpose(
                        out=t2t[:],
                        in_=t2[:].rearrange("p t ah b ap -> p (t ah b ap)")
                        .bitcast(u32))
                    # MM3 burst: contract b -> z3 [(g,B),(t,Ah,C,Ap)]
                    t2tv = t2t[:].bitcast(bf16).rearrange(
                        "p (t f) -> p t f", t=M, f=FF)
                    z3 = psA.tile([128, FM], f32, tag="z")
                    for t in range(M):
                        o = t * FF
                        mm(z3[:, o:o + 512], wk1t[:], t2tv[:, t, 0:512])
                        mm(z3[:, o + 512:o + 1024], wk1t[:],
                           t2tv[:, t, 512:1024])
                    # E3: cast evac (ACT cols [0:ca], DVE rest)
                    yv = Y[:, s0:s0 + M].rearrange("p t f -> p (t f)")
                    ca = FM - E3_DVE
                    if ca > 0:
                        nc.scalar.copy(out=yv[:, 0:ca], in_=z3[:, 0:ca])
                    if E3_DVE > 0:
                        nc.vector.tensor_copy(
                            out=yv[:, ca:FM], in_=z3[:, ca:FM])
                nc.sync.dma_start(
                    out=ys[st * SS:(st + 1) * SS].rearrange("t p f -> p t f"),
                    in_=Y[:])

    nc.compile()
    _CACHE[key] = nc
    return nc


def _kron4(w, np_dtype):
    return np.kron(np.eye(G, dtype=np.float32),
                   np.asarray(w, np.float32)).astype(np_dtype)


def make_in_maps(x, w0, w1, w2):
    import ml_dtypes
    bf = np.dtype(ml_dtypes.bfloat16)
    x = np.ascontiguousarray(np.asarray(x, np.float32).reshape(-1))
    assert x.size == BATCH * ELEM
    shards = x.reshape(N_CORES, NT, 128, FF).astype(bf)
    wk0 = _kron4(w0, bf)
    wk1 = _kron4(w1, bf)
    wk2 = _kron4(w2, bf)
    return [
        {"xs": shards[i], "wk0": wk0, "wk1": wk1, "wk2": wk2}
        for i in range(N_CORES)
    ]


def kernel(x, w0, w1, w2, trace=False):
    from concourse.bass_utils import run_bass_kernel_spmd

    nc = build_program()
    in_maps = make_in_maps(x, w0, w1, w2)
    res = run_bass_kernel_spmd(nc, in_maps, core_ids=list(range(N_CORES)),
                               trace=trace)
    # ys: [NT, (g, B), (Ah, C, Ap)] per core -> out[batch, A, B, C]
    ys = np.stack([res.results[i]["ys"] for i in range(N_CORES)])
    ys = ys.reshape(N_CORES, NT, G, F, 16, F, 2)  # [core,t,g,B,Ah,C,Ap]
    out = ys.transpose(0, 1, 2, 4, 6, 3, 5)       # [core,t,g,Ah,Ap,B,C]
    out = np.ascontiguousarray(out).astype(np.float32).reshape(-1)
    if trace:
        return out, res
    return out


# revision 20
# speedup vs baseline: 2.0203x; 2.0203x over previous
"""Tucker-style 3-mode contraction kernel for Trainium2 (8 NeuronCores).

Problem: x [1024*32*32*32] fp32, w0/w1/w2 [32,32] fp32.
  out[B,A,Bb,C] = sum_{a,b,c} x[B,a,b,c] w0[a,A] w1[b,Bb] w2[c,C]

v10 = v8 structure + knobs. bf16 I/O (host casts), contract a -> c -> b.
Per core: 128 batch elems as 32 sub-tiles of [128 p = (g4, mode32), 1024 f].
Stationary weights kron(I4, w) [128,128] bf16.

Per sub-tile:
  DMA in  X [(g,a),(b,c)] bf16                 (natural layout)
  MM1 wk0 -> ps1 [(g,A),(b,c)] f32
  T1  DVE f32 stream-transpose PSUM->SBUF -> t1 [(g,c),(b,A)] f32
  MM2 wk2, rhs = high bf16 halves of t1 (stride-2 AP, truncation cast)
      -> ps2 [(g,C),(b,A)] f32
  E2  ACT reorder+cast -> t2 [(g,C),(Ah,b,Ap)] bf16    (A = 2*Ah+Ap)
  T2  DVE u32-pair transpose -> t2t [(g,b),(Ah,C,Ap)] bf16
  MM3 wk1 -> ps3 [(g,B),(Ah,C,Ap)] f32
  E3  cast evac -> Y bf16 (ACT cols [0:FF-E3_DVE], DVE rest)
  DMA out                                       (host unscrambles Ah/Ap)
"""

import os

import numpy as np

N_CORES = 8
BATCH = 1024
F = 32
FF = F * F  # 1024
ELEM = F * FF  # 32768
B_PER_CORE = BATCH // N_CORES  # 128
G = 4
NT = B_PER_CORE // G  # 32 sub-tiles per core
SS = 4  # sub-tiles per super-tile (DMA batch)
NST = NT // SS

MM_N = int(os.environ.get("KERNEL_MM_N", "512"))  # cols per matmul instr
PS = os.environ.get("KERNEL_PS", "112")  # psum pool bufs for ps1/ps2/ps3
E3_DVE = int(os.environ.get("KERNEL_E3_DVE", "0"))  # E3 cols on DVE
SB_BUFS = int(os.environ.get("KERNEL_SB_BUFS", "2"))  # sbuf pool depth

X_DTYPE = Z_DTYPE = "bfloat16"  # for test.py printout compat

_CACHE = {}


def build_program():
    key = (MM_N, PS, E3_DVE, SB_BUFS)
    if key in _CACHE:
        return _CACHE[key]

    import concourse.bacc as bacc
    import concourse.mybir as mybir
    import concourse.tile as tile

    f32 = mybir.dt.float32
    u32 = mybir.dt.uint32
    bf16 = mybir.dt.bfloat16

    nc = bacc.Bacc("TRN2", target_bir_lowering=False, debug=False,
                   num_devices=N_CORES)

    xs = nc.dram_tensor("xs", [NT, 128, FF], bf16, kind="ExternalInput")
    wk0 = nc.dram_tensor("wk0", [128, 128], bf16, kind="ExternalInput")
    wk1 = nc.dram_tensor("wk1", [128, 128], bf16, kind="ExternalInput")
    wk2 = nc.dram_tensor("wk2", [128, 128], bf16, kind="ExternalInput")
    ys = nc.dram_tensor("ys", [NT, 128, FF], bf16, kind="ExternalOutput")

    def mms(out_tile, lhsT, rhs_slices):
        """Issue matmuls of width MM_N covering FF columns."""
        for o in range(0, FF, MM_N):
            nc.tensor.matmul(out_tile[:, o:o + MM_N], lhsT,
                             rhs_slices(o, o + MM_N), start=True, stop=True)

    b1, b2, b3 = (int(c) for c in PS)

    with tile.TileContext(nc) as tc:
        with (
            tc.tile_pool(name="consts", bufs=1) as cpool,
            tc.tile_pool(name="xp", bufs=3) as xp,
            tc.tile_pool(name="t1p", bufs=SB_BUFS) as t1p,
            tc.tile_pool(name="t2p", bufs=SB_BUFS) as t2p,
            tc.tile_pool(name="t2tp", bufs=SB_BUFS) as t2tp,
            tc.tile_pool(name="yp", bufs=2) as yp,
            tc.tile_pool(name="ps1", bufs=b1, space="PSUM") as ps1,
            tc.tile_pool(name="ps2", bufs=b2, space="PSUM") as ps2,
            tc.tile_pool(name="ps3", bufs=b3, space="PSUM") as ps3,
        ):
            wk0t = cpool.tile([128, 128], bf16)
            wk1t = cpool.tile([128, 128], bf16)
            wk2t = cpool.tile([128, 128], bf16)
            nc.sync.dma_start(out=wk0t[:], in_=wk0[:])
            nc.sync.dma_start(out=wk1t[:], in_=wk1[:])
            nc.sync.dma_start(out=wk2t[:], in_=wk2[:])

            for st in range(NST):
                X = xp.tile([128, SS, FF], bf16, tag="X")
                nc.sync.dma_start(
                    out=X[:],
                    in_=xs[st * SS:(st + 1) * SS].rearrange("t p f -> p t f"))
                Y = yp.tile([128, SS, FF], bf16, tag="Y")
                for s in range(SS):
                    # MM1: contract a -> ps1 [(g,A),(b,c)]
                    z1 = ps1.tile([128, FF], f32, tag="z1")
                    mms(z1, wk0t[:], lambda lo, hi: X[:, s, lo:hi])
                    # T1: f32 psum->sbuf transpose -> [(g,c),(b,A)] f32
                    t1 = t1p.tile([128, FF], f32, tag="t1")
                    nc.vector.transpose(out=t1[:], in_=z1[:])
                    # MM2: rhs = high bf16 halves of t1 (truncation cast)
                    t1v = t1[:].bitcast(bf16).rearrange(
                        "p (b a two) -> p b a two", b=F, a=F, two=2)
                    z2 = ps2.tile([128, FF], f32, tag="z2")
                    mms(z2, wk2t[:],
                        lambda lo, hi: t1v[:, lo // F:hi // F, :, 1])
                    # E2: reorder+cast -> t2 [(g,C), (Ah, b, Ap)]
                    t2 = t2p.tile([128, 16, F, 2], bf16, tag="t2")
                    nc.scalar.copy(
                        out=t2[:],
                        in_=z2[:].rearrange(
                            "p (b ah ap) -> p ah b ap", b=F, ah=16, ap=2))
                    # T2: u32 pair transpose -> [(g,b), (Ah, C, Ap)]
                    t2t = t2tp.tile([128, 512], u32, tag="t2t")
                    nc.vector.transpose(
                        out=t2t[:],
                        in_=t2[:].rearrange("p ah b ap -> p (ah b ap)")
                        .bitcast(u32))
                    # MM3: contract b -> ps3 [(g,B), (Ah, C, Ap)]
                    t2tv = t2t[:].bitcast(bf16)
                    z3 = ps3.tile([128, FF], f32, tag="z3")
                    mms(z3, wk1t[:], lambda lo, hi: t2tv[:, lo:hi])
                    # E3: cast evac (ACT cols [0:ca], DVE rest)
                    ca = FF - E3_DVE
                    if ca > 0:
                        nc.scalar.copy(out=Y[:, s, 0:ca], in_=z3[:, 0:ca])
                    if E3_DVE > 0:
                        nc.vector.tensor_copy(
                            out=Y[:, s, ca:FF], in_=z3[:, ca:FF])
                nc.sync.dma_start(
                    out=ys[st * SS:(st + 1) * SS].rearrange("t p f -> p t f"),
                    in_=Y[:])

    nc.compile()
    _CACHE[key] = nc
    return nc


def _kron4(w, np_dtype):
    return np.kron(np.eye(G, dtype=np.float32),
                   np.asarray(w, np.float32)).astype(np_dtype)


def make_in_maps(x, w0, w1, w2):
    import ml_dtypes
    bf = np.dtype(ml_dtypes.bfloat16)
    x = np.ascontiguousarray(np.asarray(x, np.float32).reshape(-1))
    assert x.size == BATCH * ELEM
    shards = x.reshape(N_CORES, NT, 128, FF).astype(bf)
    wk0 = _kron4(w0, bf)
    wk1 = _kron4(w1, bf)
    wk2 = _kron4(w2, bf)
    return [
        {"xs": shards[i], "wk0": wk0, "wk1": wk1, "wk2": wk2}
        for i in range(N_CORES)
    ]


def kernel(x, w0, w1, w2, trace=False):
    from concourse.bass_utils import run_bass_kernel_spmd

    nc = build_program()
    in_maps = make_in_maps(x, w0, w1, w2)
    res = run_bass_kernel_spmd(nc, in_maps, core_ids=list(range(N_CORES)),
                               trace=trace)
    # ys: [NT, (g, B), (Ah, C, Ap)] per core -> out[batch, A, B, C]
    ys = np.stack([res.results[i]["ys"] for i in range(N_CORES)])
    ys = ys.reshape(N_CORES, NT, G, F, 16, F, 2)  # [core,t,g,B,Ah,C,Ap]
    out = ys.transpose(0, 1, 2, 4, 6, 3, 5)       # [core,t,g,Ah,Ap,B,C]
    out = np.ascontiguousarray(out).astype(np.float32).reshape(-1)
    if trace:
        return out, res
    return out
